# revision 17
# baseline (speedup 1.0000x reference)
"""GRU decoder (nn_Decoder2) Trainium2 Bass kernel.

Architecture (per core, pure data parallelism over batch):
  B_shard = 4096 batch rows per core, 8 chunks of 512.
  Layout: "gate-major matmul, partition-stacked elementwise".
  For each time step and each chunk c (4 chunks per supergroup g):
    - 4 matmuls (M=32 each, fp32r, N=512), one per gate block
      {z, r, xh, mh}, writing PSUM at partition offset 32*(c%4) so that a
      supergroup's 4 chunks stack into full 128-partition PSUM tiles:
        zr_psum[g] [128, 1024]: z bank (free 0:512), r bank (512:1024)
        xm_psum[g] [128, 1024]: xh bank, mh bank
    - elementwise on full 128 partitions:
        sigmoid(zr) -> zrs ; t2 = r*mh ; t3 = t2+xh ; hh = tanh(t3)
        d = h-hh ; e = z*d ; h' = e+hh  (h' overwrites h_s[g])
    - dense output folded: out_t = h' @ dense_w (+bias on host); computed as
      4 M=1 matmuls into freed PSUM partition-0 rows, DMA'd to DRAM.
    - h' DMA-scattered (SBUF->SBUF) into each chunk's rhs tile for step t+1.
  The prev_out -> next-x dependency is folded into the recurrent weights:
      mx_{t+1} = feat@kf + (h @ dense_w + db) * k0 + bx
               = feat@kf + h @ (dense_w k0) + (bx + db k0)
  so no per-step dense->input roundtrip is needed.
"""

import numpy as np

B, T, F, H = 32768, 48, 16, 32
NCORES = 8
BS = B // NCORES            # 4096 batch per core
CK = 512                    # chunk batch size
NCHUNK = BS // CK           # 8 chunks
NSG = 2                     # supergroups
SGC = NCHUNK // NSG         # 4 chunks per supergroup

_CACHE = {}


def _prep_weights(kernel, recurrent_kernel, bias_x, bias_h, dense_w, dense_b):
    """Fold dense layer + biases into the per-gate weight matrices.

    Returns wfh [49,128] (steps t>=1), w0 [50,128] (step 0), wd [128,1].
    Gate column blocks: 0:32 z, 32:64 r, 64:96 xh, 96:128 mh.
    Row layout of wfh: 0:16 feat, 16 bias(ones row), 17:49 h.
    Row layout of w0:  0:16 feat, 16 bias, 17:49 h(=init_state), 49 prev_out(=y0).
    """
    kd = kernel.astype(np.float64)
    rkd = recurrent_kernel.astype(np.float64)
    bxd = bias_x.astype(np.float64)
    bhd = bias_h.astype(np.float64)
    dwd = dense_w.astype(np.float64)[:, 0]          # [32]
    dbd = float(dense_b.astype(np.float64)[0])

    k0 = kd[0]                                      # [96] prev_out row
    kf = kd[1:]                                     # [16, 96]
    dwk0 = np.outer(dwd, k0)                        # [32, 96] dense fold

    # Row layout (matches rhs tile): 0:32 h, 32:48 feat, 48 ones/bias, 49 y0
    wfh = np.zeros((50, 128), np.float64)
    w0 = np.zeros((50, 128), np.float64)
    for gi, (lo, hi) in enumerate([(0, 32), (32, 64), (64, 96), (96, 128)]):
        src = [slice(0, 32), slice(32, 64), slice(64, 96), slice(64, 96)][gi]
        if gi < 3:   # z, r, xh take feat + prev_out terms
            wfh[32:48, lo:hi] = kf[:, src]
            w0[32:48, lo:hi] = kf[:, src]
            w0[49, lo:hi] = k0[src]
        if gi < 2:   # z, r: mx+mh summed
            wfh[48, lo:hi] = bxd[src] + bhd[src] + dbd * k0[src]
            wfh[0:32, lo:hi] = rkd[:, src] + dwk0[:, src]
            w0[48, lo:hi] = bxd[src] + bhd[src]
            w0[0:32, lo:hi] = rkd[:, src]
        elif gi == 2:  # xh: mx only
            wfh[48, lo:hi] = bxd[src] + dbd * k0[src]
            wfh[0:32, lo:hi] = dwk0[:, src]
            w0[48, lo:hi] = bxd[src]
            # w0 h rows stay zero (no h term in mx at t=0)
        else:        # mh: h only
            wfh[48, lo:hi] = bhd[src]
            wfh[0:32, lo:hi] = rkd[:, src]
            w0[48, lo:hi] = bhd[src]
            w0[0:32, lo:hi] = rkd[:, src]

    wd = np.tile(dwd, 4)[:, None]                   # [128, 1]
    return (wfh.astype(np.float32), w0.astype(np.float32),
            np.ascontiguousarray(wd.astype(np.float32)))


def _build_module(n_steps=T):
    import concourse.bacc as bacc
    import concourse.mybir as mybir
    import concourse.tile as tile
    from contextlib import ExitStack

    f32 = mybir.dt.float32
    f16 = mybir.dt.float16
    AF = mybir.ActivationFunctionType

    nc = bacc.Bacc("TRN2")
    feat = nc.dram_tensor("feat", [n_steps, F + 2, BS], f16, kind="ExternalInput")
    h0 = nc.dram_tensor("h0", [H, BS], f16, kind="ExternalInput")
    wfh_d = nc.dram_tensor("wfh", [50, 128], f16, kind="ExternalInput")
    w0_d = nc.dram_tensor("w0", [50, 128], f16, kind="ExternalInput")
    wd_d = nc.dram_tensor("wd", [128, 1], f16, kind="ExternalInput")
    out = nc.dram_tensor("out", [n_steps, BS], f32, kind="ExternalOutput")

    with tile.TileContext(nc) as tc, ExitStack() as ctx:
        wpool = ctx.enter_context(tc.tile_pool(name="weights", bufs=1))
        xpool = ctx.enter_context(tc.tile_pool(name="xc", bufs=1))
        hpool = ctx.enter_context(tc.tile_pool(name="hs", bufs=1))
        ew = ctx.enter_context(tc.tile_pool(name="ew", bufs=2))
        ppool = ctx.enter_context(tc.tile_pool(name="psum", bufs=1, space="PSUM"))

        wfh_s = wpool.tile([50, 128], f16, tag="wfh")
        w0_s = wpool.tile([50, 128], f16, tag="w0")
        wd_s = wpool.tile([128, 1], f16, tag="wd")
        nc.sync.dma_start(wfh_s[:, :], wfh_d[:, :])
        nc.sync.dma_start(w0_s[:, :], w0_d[:, :])
        nc.sync.dma_start(wd_s[:, :], wd_d[:, :])

        # Per-chunk rhs tiles [50, 512]: rows 0:32 h, 32:48 feat, 48 ones, 49 y0
        xc = []
        for c in range(NCHUNK):
            t_ = xpool.tile([50, CK], f16, tag=f"xc{c}", name=f"xc{c}")
            sl = slice(c * CK, (c + 1) * CK)
            nc.sync.dma_start(t_[0:32, :], h0[:, sl])
            nc.sync.dma_start(t_[32:50, :], feat[0, :, sl])
            xc.append(t_)

        # Stacked state tiles per supergroup [128, 512]
        h_s = []
        for g in range(NSG):
            t_ = hpool.tile([128, CK], f16, tag=f"hs{g}", name=f"hs{g}")
            for ci in range(SGC):
                c = g * SGC + ci
                nc.sync.dma_start(t_[32 * ci:32 * ci + 32, :],
                                  h0[:, c * CK:(c + 1) * CK])
            h_s.append(t_)

        zr_ps = [ppool.tile([128, 2 * CK], f32, tag=f"zr{g}", name=f"zr{g}")
                 for g in range(NSG)]
        xm_ps = [ppool.tile([128, 2 * CK], f32, tag=f"xm{g}", name=f"xm{g}")
                 for g in range(NSG)]

        for t in range(n_steps):
            lhs = w0_s if t == 0 else wfh_s
            for g in range(NSG):
                zr, xm = zr_ps[g], xm_ps[g]
                for ci in range(SGC):
                    c = g * SGC + ci
                    rhs = xc[c][0:50, :]
                    p0 = 32 * ci
                    tp = (0, p0)
                    nc.tensor.matmul(zr[p0:p0 + 32, 0:CK],
                                     lhsT=lhs[0:50, 0:32], rhs=rhs,
                                     start=True, stop=True, tile_position=tp)
                    nc.tensor.matmul(zr[p0:p0 + 32, CK:2 * CK],
                                     lhsT=lhs[0:50, 32:64], rhs=rhs,
                                     start=True, stop=True, tile_position=tp)
                    nc.tensor.matmul(xm[p0:p0 + 32, 0:CK],
                                     lhsT=lhs[0:50, 64:96], rhs=rhs,
                                     start=True, stop=True, tile_position=tp)
                    nc.tensor.matmul(xm[p0:p0 + 32, CK:2 * CK],
                                     lhsT=lhs[0:50, 96:128], rhs=rhs,
                                     start=True, stop=True, tile_position=tp)

                zrs = ew.tile([128, 2 * CK], f16, tag=f"zrs{g}", name=f"zrs{g}_{t}")
                t2s = ew.tile([128, CK], f16, tag=f"t2s{g}", name=f"t2s{g}_{t}")
                t3s = ew.tile([128, CK], f16, tag=f"t3s{g}", name=f"t3s{g}_{t}")
                hhs = ew.tile([128, CK], f16, tag=f"hhs{g}", name=f"hhs{g}_{t}")
                ds = ew.tile([128, CK], f16, tag=f"ds{g}", name=f"ds{g}_{t}")
                es = ew.tile([128, CK], f16, tag=f"es{g}", name=f"es{g}_{t}")

                nc.scalar.activation(zrs[:, :], zr[:, :], AF.Sigmoid)
                nc.vector.tensor_mul(t2s[:, :], zrs[:, CK:2 * CK], xm[:, CK:2 * CK])
                nc.vector.tensor_add(t3s[:, :], t2s[:, :], xm[:, 0:CK])
                nc.scalar.activation(hhs[:, :], t3s[:, :], AF.Tanh)
                nc.vector.tensor_sub(ds[:, :], h_s[g][:, :], hhs[:, :])
                nc.vector.tensor_mul(es[:, :], zrs[:, 0:CK], ds[:, :])
                nc.vector.tensor_add(h_s[g][:, :], es[:, :], hhs[:, :])

                # dense: out_t[c] = h'[c] . wd -> rows {0,32,64,96} of the
                # freed M region, evacuated via one ACT copy then 4 small DMAs
                for ci in range(SGC):
                    nc.tensor.matmul(xm[32 * ci:32 * ci + 1, CK:2 * CK],
                                     lhsT=wd_s[32 * ci:32 * ci + 32, 0:1],
                                     rhs=h_s[g][32 * ci:32 * ci + 32, :],
                                     start=True, stop=True,
                                     tile_position=(32 * ci, 32 * ci))
                dsb = ew.tile([128, CK], f32, tag=f"dsb{g}", name=f"dsb{g}_{t}")
                nc.scalar.copy(dsb[:, :], xm[:, CK:2 * CK])
                gb = g * SGC * CK
                for ci in range(SGC):
                    nc.sync.dma_start(
                        out[t:t + 1, gb + ci * CK:gb + (ci + 1) * CK],
                        dsb[32 * ci:32 * ci + 1, :])

                if t < n_steps - 1:
                    for ci in range(SGC):
                        c = g * SGC + ci
                        sl = slice(c * CK, (c + 1) * CK)
                        nc.sync.dma_start(xc[c][0:32, :],
                                          h_s[g][32 * ci:32 * ci + 32, :])
                        nc.sync.dma_start(xc[c][32:50, :], feat[t + 1, :, sl])
    nc.compile()
    return nc


def _host_prep(inputs, n_steps=T):
    """Shard + transpose inputs host-side. Returns (in_maps, dense_b)."""
    dfeat = np.asarray(inputs["decoder_feature"], np.float32)
    y0 = np.asarray(inputs["decoder_init_input"], np.float32)
    h0 = np.asarray(inputs["init_state"], np.float32)
    wfh, w0, wd = _prep_weights(
        np.asarray(inputs["kernel"], np.float32),
        np.asarray(inputs["recurrent_kernel"], np.float32),
        np.asarray(inputs["bias_x"], np.float32),
        np.asarray(inputs["bias_h"], np.float32),
        np.asarray(inputs["dense_w"], np.float32),
        np.asarray(inputs["dense_b"], np.float32),
    )
    def one(sl):
        nb = sl.stop - sl.start
        fx = np.zeros((n_steps, F + 2, nb), np.float32)
        fx[:, 0:F, :] = dfeat[sl, :n_steps].transpose(1, 2, 0)
        fx[:, F, :] = 1.0                      # ones/bias plane
        fx[0, F + 1, :] = y0[sl, 0]            # y0 plane (t=0 only)
        return {
            "feat": np.ascontiguousarray(fx).astype(np.float16),   # [T, F+2, BS]
            "h0": np.ascontiguousarray(h0[sl].T).astype(np.float16),
            "wfh": wfh.astype(np.float16),
            "w0": w0.astype(np.float16),
            "wd": wd.astype(np.float16),
        }

    in_maps = [one(slice(i * BS, (i + 1) * BS)) for i in range(NCORES)]
    return in_maps, float(np.asarray(inputs["dense_b"], np.float64)[0])


def _host_prep_single(inputs, n_steps=T):
    """Single-core in_map for a batch of exactly BS rows (testing)."""
    assert inputs["init_state"].shape[0] == BS
    saved = globals()["NCORES"]
    try:
        globals()["NCORES"] = 1
        maps, db = _host_prep(inputs, n_steps)
    finally:
        globals()["NCORES"] = saved
    return maps[0], db


def run(inputs, trace=False, n_steps=T, **spmd_kwargs):
    """Run on the 8 NeuronCores; returns (out [B,T,1] fp32, BassKernelResults)."""
    from concourse.bass_utils import run_bass_kernel_spmd

    key = n_steps
    if key not in _CACHE:
        _CACHE[key] = _build_module(n_steps)
    nc = _CACHE[key]
    in_maps, db = _host_prep(inputs, n_steps)
    res = run_bass_kernel_spmd(nc, in_maps, list(range(NCORES)),
                               trace=trace, **spmd_kwargs)
    outs = np.concatenate([np.asarray(r["out"]) for r in res.results], axis=1)
    full = (outs.T[:, :, None] + np.float32(db)).astype(np.float32)
    return full, res


def kernel(**inputs) -> np.ndarray:
    out, _ = run(inputs, trace=False)
    return out


# revision 19
# speedup vs baseline: 1.4538x; 1.4538x over previous
"""GRU decoder (nn_Decoder2) Trainium2 Bass kernel.

Architecture (per core, pure data parallelism over batch):
  B_shard = 4096 batch rows per core, 8 chunks of 512.
  Layout: "gate-major matmul, partition-stacked elementwise".
  For each time step and each chunk c (4 chunks per supergroup g):
    - 4 matmuls (M=32 each, fp32r, N=512), one per gate block
      {z, r, xh, mh}, writing PSUM at partition offset 32*(c%4) so that a
      supergroup's 4 chunks stack into full 128-partition PSUM tiles:
        zr_psum[g] [128, 1024]: z bank (free 0:512), r bank (512:1024)
        xm_psum[g] [128, 1024]: xh bank, mh bank
    - elementwise on full 128 partitions:
        sigmoid(zr) -> zrs ; t2 = r*mh ; t3 = t2+xh ; hh = tanh(t3)
        d = h-hh ; e = z*d ; h' = e+hh  (h' overwrites h_s[g])
    - dense output folded: out_t = h' @ dense_w (+bias on host); computed as
      4 M=1 matmuls into freed PSUM partition-0 rows, DMA'd to DRAM.
    - h' DMA-scattered (SBUF->SBUF) into each chunk's rhs tile for step t+1.
  The prev_out -> next-x dependency is folded into the recurrent weights:
      mx_{t+1} = feat@kf + (h @ dense_w + db) * k0 + bx
               = feat@kf + h @ (dense_w k0) + (bx + db k0)
  so no per-step dense->input roundtrip is needed.
"""

import numpy as np

B, T, F, H = 32768, 48, 16, 32
NCORES = 8
BS = B // NCORES            # 4096 batch per core
CK = 512                    # chunk batch size
NCHUNK = BS // CK           # 8 chunks
NSG = 2                     # supergroups
SGC = NCHUNK // NSG         # 4 chunks per supergroup

_CACHE = {}


def _prep_weights(kernel, recurrent_kernel, bias_x, bias_h, dense_w, dense_b):
    """Fold dense layer + biases into the per-gate weight matrices.

    Returns wfh [49,128] (steps t>=1), w0 [50,128] (step 0), wd [128,1].
    Gate column blocks: 0:32 z, 32:64 r, 64:96 xh, 96:128 mh.
    Row layout of wfh: 0:16 feat, 16 bias(ones row), 17:49 h.
    Row layout of w0:  0:16 feat, 16 bias, 17:49 h(=init_state), 49 prev_out(=y0).
    """
    kd = kernel.astype(np.float64)
    rkd = recurrent_kernel.astype(np.float64)
    bxd = bias_x.astype(np.float64)
    bhd = bias_h.astype(np.float64)
    dwd = dense_w.astype(np.float64)[:, 0]          # [32]
    dbd = float(dense_b.astype(np.float64)[0])

    k0 = kd[0]                                      # [96] prev_out row
    kf = kd[1:]                                     # [16, 96]
    dwk0 = np.outer(dwd, k0)                        # [32, 96] dense fold

    # Row layout (matches rhs tile): 0:32 h, 32:48 feat, 48 ones/bias, 49 y0
    wfh = np.zeros((50, 128), np.float64)
    w0 = np.zeros((50, 128), np.float64)
    for gi, (lo, hi) in enumerate([(0, 32), (32, 64), (64, 96), (96, 128)]):
        src = [slice(0, 32), slice(32, 64), slice(64, 96), slice(64, 96)][gi]
        if gi < 3:   # z, r, xh take feat + prev_out terms
            wfh[32:48, lo:hi] = kf[:, src]
            w0[32:48, lo:hi] = kf[:, src]
            w0[49, lo:hi] = k0[src]
        if gi < 2:   # z, r: mx+mh summed
            wfh[48, lo:hi] = bxd[src] + bhd[src] + dbd * k0[src]
            wfh[0:32, lo:hi] = rkd[:, src] + dwk0[:, src]
            w0[48, lo:hi] = bxd[src] + bhd[src]
            w0[0:32, lo:hi] = rkd[:, src]
        elif gi == 2:  # xh: mx only
            wfh[48, lo:hi] = bxd[src] + dbd * k0[src]
            wfh[0:32, lo:hi] = dwk0[:, src]
            w0[48, lo:hi] = bxd[src]
            # w0 h rows stay zero (no h term in mx at t=0)
        else:        # mh: h only
            wfh[48, lo:hi] = bhd[src]
            wfh[0:32, lo:hi] = rkd[:, src]
            w0[48, lo:hi] = bhd[src]
            w0[0:32, lo:hi] = rkd[:, src]

    wd4 = np.zeros((128, 4), np.float64)            # block-diag dense weights
    for c in range(4):
        wd4[32 * c:32 * c + 32, c] = dwd
    return (wfh.astype(np.float32), w0.astype(np.float32),
            np.ascontiguousarray(wd4.astype(np.float32)))


def _build_module(n_steps=T):
    import concourse.bacc as bacc
    import concourse.mybir as mybir
    import concourse.tile as tile
    from contextlib import ExitStack

    f32 = mybir.dt.float32
    f16 = mybir.dt.float16
    AF = mybir.ActivationFunctionType

    nc = bacc.Bacc("TRN2")
    feat = nc.dram_tensor("feat", [n_steps, F + 2, BS], f16, kind="ExternalInput")
    h0 = nc.dram_tensor("h0", [H, BS], f16, kind="ExternalInput")
    wfh_d = nc.dram_tensor("wfh", [50, 128], f16, kind="ExternalInput")
    w0_d = nc.dram_tensor("w0", [50, 128], f16, kind="ExternalInput")
    wd_d = nc.dram_tensor("wd", [128, 4], f16, kind="ExternalInput")
    out = nc.dram_tensor("out", [n_steps, BS], f32, kind="ExternalOutput")

    with tile.TileContext(nc) as tc, ExitStack() as ctx:
        wpool = ctx.enter_context(tc.tile_pool(name="weights", bufs=1))
        xpool = ctx.enter_context(tc.tile_pool(name="xc", bufs=1))
        hpool = ctx.enter_context(tc.tile_pool(name="hs", bufs=1))
        ew = ctx.enter_context(tc.tile_pool(name="ew", bufs=2))
        ppool = ctx.enter_context(tc.tile_pool(name="psum", bufs=1, space="PSUM"))

        wfh_s = wpool.tile([50, 128], f16, tag="wfh")
        w0_s = wpool.tile([50, 128], f16, tag="w0")
        wd_s = wpool.tile([128, 4], f16, tag="wd")
        nc.sync.dma_start(wfh_s[:, :], wfh_d[:, :])
        nc.sync.dma_start(w0_s[:, :], w0_d[:, :])
        nc.sync.dma_start(wd_s[:, :], wd_d[:, :])

        # Per-chunk rhs tiles [50, 512]: rows 0:32 h, 32:48 feat, 48 ones, 49 y0
        xc = []
        for c in range(NCHUNK):
            t_ = xpool.tile([50, 4 * CK], f16, tag=f"xc{c}", name=f"xc{c}")
            sl = slice(c * CK, (c + 1) * CK)
            nc.sync.dma_start(t_[0:32, 0:CK], h0[:, sl])
            nc.sync.dma_start(
                t_[32:50, :].rearrange("r (s b) -> r s b", s=4),
                feat[0:4, :, sl].rearrange("s r b -> r s b"))
            xc.append(t_)

        # Stacked state tiles per supergroup [128, 512]
        h_s = []
        for g in range(NSG):
            t_ = hpool.tile([128, CK], f16, tag=f"hs{g}", name=f"hs{g}")
            for ci in range(SGC):
                c = g * SGC + ci
                nc.sync.dma_start(t_[32 * ci:32 * ci + 32, :],
                                  h0[:, c * CK:(c + 1) * CK])
            h_s.append(t_)

        zr_ps = [ppool.tile([128, 2 * CK], f32, tag=f"zr{g}", name=f"zr{g}")
                 for g in range(NSG)]
        xm_ps = [ppool.tile([128, 2 * CK], f32, tag=f"xm{g}", name=f"xm{g}")
                 for g in range(NSG)]

        for t in range(n_steps):
            lhs = w0_s if t == 0 else wfh_s
            slot = t % 4
            for g in range(NSG):
                zr, xm = zr_ps[g], xm_ps[g]
                outs_by_gate = [
                    (slice(0, 32), zr, 0), (slice(32, 64), zr, CK),
                    (slice(64, 96), xm, 0), (slice(96, 128), xm, CK)]
                for wsl, bank, fo in outs_by_gate:
                    for ci in range(SGC):
                        c = g * SGC + ci
                        rhs = xc[c][0:50, slot * CK:(slot + 1) * CK]
                        p0 = 32 * ci
                        nc.tensor.matmul(bank[p0:p0 + 32, fo:fo + CK],
                                         lhsT=lhs[0:50, wsl], rhs=rhs,
                                         start=True, stop=True,
                                         tile_position=(0, p0))

                zrs = ew.tile([128, 2 * CK], f16, tag=f"zrs{g}", name=f"zrs{g}_{t}")
                t2s = ew.tile([128, CK], f16, tag=f"t2s{g}", name=f"t2s{g}_{t}")
                t3s = ew.tile([128, CK], f16, tag=f"t3s{g}", name=f"t3s{g}_{t}")
                hhs = ew.tile([128, CK], f16, tag=f"hhs{g}", name=f"hhs{g}_{t}")
                ds = ew.tile([128, CK], f16, tag=f"ds{g}", name=f"ds{g}_{t}")
                es = ew.tile([128, CK], f16, tag=f"es{g}", name=f"es{g}_{t}")

                nc.scalar.activation(zrs[:, :], zr[:, :], AF.Sigmoid)
                nc.vector.tensor_mul(t2s[:, :], zrs[:, CK:2 * CK], xm[:, CK:2 * CK])
                nc.vector.tensor_add(t3s[:, :], t2s[:, :], xm[:, 0:CK])
                nc.scalar.activation(hhs[:, :], t3s[:, :], AF.Tanh)
                nc.vector.tensor_sub(ds[:, :], h_s[g][:, :], hhs[:, :])
                nc.vector.tensor_mul(es[:, :], zrs[:, 0:CK], ds[:, :])
                nc.vector.tensor_add(h_s[g][:, :], es[:, :], hhs[:, :])

                # dense: one K=128 block-diag matmul -> rows 0:4 of the
                # freed M region; ACT evac; one DMA out per supergroup
                nc.tensor.matmul(xm[0:4, CK:2 * CK],
                                 lhsT=wd_s[0:128, 0:4],
                                 rhs=h_s[g][:, :],
                                 start=True, stop=True, tile_position=(0, 0))
                dsb = ew.tile([4, CK], f32, tag=f"dsb{g}", name=f"dsb{g}_{t}")
                nc.scalar.copy(dsb[:, :], xm[0:4, CK:2 * CK])
                gb = g * SGC * CK
                nc.sync.dma_start(
                    out[t, gb:gb + 4 * CK].rearrange("(c b) -> c b", c=4),
                    dsb[:, :])

                if t < n_steps - 1:
                    nslot = (t + 1) % 4
                    for ci in range(SGC):
                        c = g * SGC + ci
                        sl = slice(c * CK, (c + 1) * CK)
                        nc.sync.dma_start(
                            xc[c][0:32, nslot * CK:(nslot + 1) * CK],
                            h_s[g][32 * ci:32 * ci + 32, :])
                        if t % 4 == 3:
                            t0 = t + 1
                            t1 = min(t0 + 4, n_steps)
                            nc.sync.dma_start(
                                xc[c][32:50, 0:(t1 - t0) * CK].rearrange(
                                    "r (s b) -> r s b", s=t1 - t0),
                                feat[t0:t1, :, sl].rearrange("s r b -> r s b"))
    nc.compile()
    return nc


def _host_prep(inputs, n_steps=T):
    """Shard + transpose inputs host-side. Returns (in_maps, dense_b)."""
    dfeat = np.asarray(inputs["decoder_feature"], np.float32)
    y0 = np.asarray(inputs["decoder_init_input"], np.float32)
    h0 = np.asarray(inputs["init_state"], np.float32)
    wfh, w0, wd = _prep_weights(
        np.asarray(inputs["kernel"], np.float32),
        np.asarray(inputs["recurrent_kernel"], np.float32),
        np.asarray(inputs["bias_x"], np.float32),
        np.asarray(inputs["bias_h"], np.float32),
        np.asarray(inputs["dense_w"], np.float32),
        np.asarray(inputs["dense_b"], np.float32),
    )
    def one(sl):
        nb = sl.stop - sl.start
        fx = np.zeros((n_steps, F + 2, nb), np.float32)
        fx[:, 0:F, :] = dfeat[sl, :n_steps].transpose(1, 2, 0)
        fx[:, F, :] = 1.0                      # ones/bias plane
        fx[0, F + 1, :] = y0[sl, 0]            # y0 plane (t=0 only)
        return {
            "feat": np.ascontiguousarray(fx).astype(np.float16),   # [T, F+2, BS]
            "h0": np.ascontiguousarray(h0[sl].T).astype(np.float16),
            "wfh": wfh.astype(np.float16),
            "w0": w0.astype(np.float16),
            "wd": wd.astype(np.float16),
        }

    in_maps = [one(slice(i * BS, (i + 1) * BS)) for i in range(NCORES)]
    return in_maps, float(np.asarray(inputs["dense_b"], np.float64)[0])


def _host_prep_single(inputs, n_steps=T):
    """Single-core in_map for a batch of exactly BS rows (testing)."""
    assert inputs["init_state"].shape[0] == BS
    saved = globals()["NCORES"]
    try:
        globals()["NCORES"] = 1
        maps, db = _host_prep(inputs, n_steps)
    finally:
        globals()["NCORES"] = saved
    return maps[0], db


def run(inputs, trace=False, n_steps=T, **spmd_kwargs):
    """Run on the 8 NeuronCores; returns (out [B,T,1] fp32, BassKernelResults)."""
    from concourse.bass_utils import run_bass_kernel_spmd

    key = n_steps
    if key not in _CACHE:
        _CACHE[key] = _build_module(n_steps)
    nc = _CACHE[key]
    in_maps, db = _host_prep(inputs, n_steps)
    res = run_bass_kernel_spmd(nc, in_maps, list(range(NCORES)),
                               trace=trace, **spmd_kwargs)
    outs = np.concatenate([np.asarray(r["out"]) for r in res.results], axis=1)
    full = (outs.T[:, :, None] + np.float32(db)).astype(np.float32)
    return full, res


def kernel(**inputs) -> np.ndarray:
    out, _ = run(inputs, trace=False)
    return out
